# revision 38
# baseline (speedup 1.0000x reference)
"""Trainium2 Bass kernel for GroupNorm + single-head self-attention block.

Reference computation (per batch b):
    xn = GroupNorm(x; 32 groups over (L, C/32)) * gn_scale + gn_bias
    q, k, v = xn@wq+bq, xn@wk+bk, xn@wv+bv
    out = softmax(q k^T / sqrt(C)) v @ wo + bo + x

Sharding: 8 cores = 4 batches x 2 query-halves. Each core receives its
batch's [L=4096, C=512] slice ROTATED so that its 2048 query rows are
always rows 0..2047 (attention and GN stats are invariant to key/value
ordering), which keeps the program SPMD-identical across cores.

On-core dataflow (fp8 DoubleRow matmuls everywhere heavy, fp32 accum):
  - x is pre-cast to bf16 on the host (stats, transposes and the
    residual all tolerate bf16), halving the input stream to 4MB.
  - GN is folded into the projections: A[c]=rstd[g]*gn_scale[c],
    B[c]=gn_bias[c]-mean[g]*A[c]; wq'=S*A (.) wq (row scaling),
    b'q = S*(B@wq + bq), with S=16 a power-of-2 pre-scale that keeps
    the fp8 weights out of the subnormal range. wo'=wo*64/S with the
    O~ accumulator scaled by 1/64 in its PSUM->fp8 copy.
  - Phase X: x streams in bf16, is cast to fp8 (xball, SBUF-resident);
    GN stats accumulate via fp8 DoubleRow ones-matmuls (x and fp8
    squares from ACT). The 128 PE transposes of xball -> xT [C, L] are
    emitted AFTER the stats loop so they fill the PE while the serial
    GN-stats chain (DVE/ACT) runs, keeping the HAM clock-gate warm.
  - Q^T/K^T PSUM tiles drain alternately through ACT (Identity +
    per-partition bias column) and DVE, because either engine alone is
    slower than the PE producing the tiles. V is computed BIAS-FREE:
    softmax@(v+b)@wo = softmax@v@wo + b@wo, so b'v@wo folds exactly
    into the output bias and V's drains are plain ACT/DVE copies.
  - Attention is software-pipelined one key-pair ahead ACROSS query
    blocks: the PE queue order is scores(next pair), zsum(deferred),
    AV(current), with each block's epilogue emitted after the next
    block's first scores, so the strict-FIFO engine queue never
    head-of-line blocks on ACT exps or DVE drains.
  - Row sums Z come from ones-matmuls over DVE pair-sums of exp tiles,
    deferred 2 pairs; 1/Z is taken on a [128,4] column layout after a
    PE transpose of the Z row; the final DR projection is scaled by
    1/Z and fused with bias + residual in one DVE op.
"""

import sys

sys.path.insert(0, "/opt/trn_rl_repo")

import numpy as np

B, HH, WW, C = 4, 64, 64, 512
L = HH * WW          # 4096
G = 32               # groups
GS = C // G          # 16 channels per group
EPS = 1e-6
NCORES = 8
LQ = L // 2          # 2048 query rows per core
PT = 128             # partition tile
NT = L // PT         # 32 row tiles
CCH = C // PT        # 4 channel chunks
NB = 512             # matmul moving-free block
NP = NT // 2         # 16 key pairs in attention
NLB = LQ // NB       # 4 query blocks
S = 16.0             # fp8 pre-scale folded into wq/wk/wv rows
SCALE = 1.0 / float(np.sqrt(C))
EXPB = -4.0          # constant shift inside exp (cancels in softmax)


def build_program():
    import concourse.bacc as bacc
    import concourse.bass as bass
    import concourse.mybir as mybir
    import concourse.tile as tile

    f32 = mybir.dt.float32
    bf16 = mybir.dt.bfloat16
    f8 = mybir.dt.float8e4
    AF = mybir.ActivationFunctionType
    DR = mybir.MatmulPerfMode.DoubleRow

    nc = bacc.Bacc(
        trn_type="TRN2",
        target_bir_lowering=False,
        debug=False,
        num_devices=NCORES,
    )

    x_d = nc.dram_tensor("x", [L, C], bf16, kind="ExternalInput").ap()
    gs_d = nc.dram_tensor("gn_scale", [C], f32, kind="ExternalInput").ap()
    gb_d = nc.dram_tensor("gn_bias", [C], f32, kind="ExternalInput").ap()
    w_d = {}
    b_d = {}
    for n in "qkvo":
        w_d[n] = nc.dram_tensor("w" + n, [C, C], f32, kind="ExternalInput").ap()
        b_d[n] = nc.dram_tensor("b" + n, [C], f32, kind="ExternalInput").ap()
    eg_d = nc.dram_tensor("egrp", [G, C], f32, kind="ExternalInput").ap()
    eye_d = nc.dram_tensor("eye", [PT, PT], bf16, kind="ExternalInput").ap()
    y_d = nc.dram_tensor("y", [LQ, C], f32, kind="ExternalOutput").ap()

    with tile.TileContext(nc) as tc:
        with (
            tc.tile_pool(name="persist", bufs=1) as pp,
            tc.tile_pool(name="trans", bufs=1) as tp,
            tc.tile_pool(name="dram", bufs=1, space="DRAM") as dp,
            tc.tile_pool(name="psum", bufs=1, space="PSUM") as psp,
        ):
            # ---- constants ----
            ones_col = pp.tile([PT, 1], bf16, tag="ones_col")
            nc.vector.memset(ones_col, 1.0)
            # fp8 DoubleRow ones for the stats matmuls; middle-dim byte
            # step must be 16-aligned, hence the padded [PT, 2, 16] tile
            ones82 = pp.tile([PT, 2, 16], f8, tag="ones82")
            nc.vector.memset(ones82, 1.0)
            ones_row = pp.tile([1, PT], bf16, tag="ones_row")
            nc.vector.memset(ones_row, 1.0)
            expb_t = pp.tile([PT, 1], f32, tag="expb")
            nc.vector.memset(expb_t, float(EXPB))
            one_f = pp.tile([1, 1], f32, tag="one_f")
            nc.vector.memset(one_f, 1.0)

            # nothing precedes the x stream on the sync ring
            eye_sb = pp.tile([PT, PT], bf16, tag="eye")
            nc.gpsimd.dma_start(out=eye_sb, in_=eye_d)
            # group->channel indicator matrix [G, C] (host-side constant)
            eg_sb = pp.tile([G, C], f32, tag="eg")
            nc.gpsimd.dma_start(out=eg_sb, in_=eg_d)

            # ---- persistent tensors ----
            xball = pp.tile([PT, NT, C], bf16, tag="xball")     # 4 MB
            xT = pp.tile([PT, CCH, L], f8, tag="xT")            # 2 MB
            qT = pp.tile([PT, CCH, LQ], f8, tag="qT")           # 1 MB
            kT = pp.tile([PT, CCH, L], f8, tag="kT")            # 2 MB
            v_sb = pp.tile([PT, NT, NB], f8, tag="v")           # 2 MB
            wb = {
                n: pp.tile([PT, CCH, C], f8, tag="wb" + n, name="wb_" + n)
                for n in "qkvo"
            }

            def row_to_col(row, out_sb, nm):
                """[1, n*128] row -> [128, n] column layout, via K=1 matmuls."""
                n = out_sb.shape[-1]
                ps = psp.tile(
                    [PT, n], f32, tag="ps", bufs=2, name="r2c_" + nm
                )
                for m_ in range(n):
                    nc.tensor.matmul(
                        ps[:, m_ : m_ + 1],
                        row[0:1, m_ * PT : (m_ + 1) * PT],
                        one_f,
                        start=True,
                        stop=True,
                    )
                nc.vector.tensor_copy(out_sb, ps)

            # ---- small bias/scale loads (gpsimd ring; off the x stream) ----
            bq_row = pp.tile([1, C], f32, tag="bq_row")
            nc.gpsimd.dma_start(out=bq_row, in_=b_d["q"].unsqueeze(0))
            bk_row = pp.tile([1, C], f32, tag="bk_row")
            nc.gpsimd.dma_start(out=bk_row, in_=b_d["k"].unsqueeze(0))
            bv_row = pp.tile([1, C], f32, tag="bv_row")
            nc.gpsimd.dma_start(out=bv_row, in_=b_d["v"].unsqueeze(0))
            bo_bc = pp.tile([PT, C], f32, tag="bo_bc")
            nc.gpsimd.dma_start(
                out=bo_bc, in_=b_d["o"].unsqueeze(0).broadcast_to([PT, C])
            )
            gs_row = pp.tile([1, C], f32, tag="gs_row")
            nc.gpsimd.dma_start(out=gs_row, in_=gs_d.unsqueeze(0))
            gb_row = pp.tile([1, C], f32, tag="gb_row")
            nc.gpsimd.dma_start(out=gb_row, in_=gb_d.unsqueeze(0))

            # ---- phase X: stream x (bf16), cast to fp8, GN stats ----
            sum_ps = psp.tile([1, C], f32, tag="pz")
            sq_ps = psp.tile([1, C], f32, tag="py")
            for t2 in range(NT // 2):
                xf4 = tp.tile([PT, 2, C], bf16, tag="xf4", bufs=3, name=f"xf4_{t2}")
                nc.sync.dma_start(
                    out=xf4,
                    in_=x_d[t2 * 2 * PT : (t2 + 1) * 2 * PT, :].rearrange(
                        "(i p) c -> p i c", p=PT
                    ),
                )
                for i in range(2):
                    t = 2 * t2 + i
                    nc.vector.tensor_copy(xball[:, t, :], xf4[:, i, :])
                    sq = tp.tile([PT, C], bf16, tag="sq", bufs=2)
                    nc.scalar.activation(out=sq, in_=xf4[:, i, :], func=AF.Square)
                    nc.tensor.matmul(
                        sum_ps, ones_col, xball[:, t, :],
                        start=(t == 0), stop=(t == NT - 1),
                    )
                    nc.tensor.matmul(
                        sq_ps, ones_col, sq,
                        start=(t == 0), stop=(t == NT - 1),
                    )

            # ---- weight loads: sync ring AFTER the x stream (FIFO keeps
            # them from competing with x for HBM bandwidth) ----
            wf = {}
            for n in "qkvo":
                wf[n] = tp.tile([PT, CCH, C], f32, tag="wf", bufs=3, name="wf_" + n)
                nc.sync.dma_start(
                    out=wf[n], in_=w_d[n].rearrange("(j p) c -> p j c", p=PT)
                )

            # ---- phase T: transposes, emitted after stats so they fill the
            # PE while the serial stats chain runs on DVE/ACT ----
            for t in range(NT):
                t_ps = psp.tile([PT, NB], bf16, tag="po", bufs=4, name=f"tps{t}")
                for j in range(CCH):
                    nc.tensor.transpose(
                        t_ps[:, j * PT : (j + 1) * PT],
                        xball[:, t, j * PT : (j + 1) * PT],
                        eye_sb,
                    )
                nc.vector.tensor_copy(
                    xT[:, :, t * PT : (t + 1) * PT],
                    t_ps.rearrange("p (j i) -> p j i", j=CCH),
                )

            # ---- phase S: GN stats -> A,B rows -> bounce to [128,4] ----
            s1 = tp.tile([1, G], f32, tag="small", bufs=8)
            nc.vector.reduce_sum(
                out=s1,
                in_=sum_ps.rearrange("p (g d) -> p g d", g=G),
                axis=mybir.AxisListType.X,
            )
            s2 = tp.tile([1, G], f32, tag="small", bufs=8)
            nc.vector.reduce_sum(
                out=s2,
                in_=sq_ps.rearrange("p (g d) -> p g d", g=G),
                axis=mybir.AxisListType.X,
            )
            inv_n = 1.0 / float(L * GS)
            mean = tp.tile([1, G], f32, tag="small", bufs=8)
            nc.vector.tensor_scalar_mul(mean, s1, inv_n)
            ex2 = tp.tile([1, G], f32, tag="small", bufs=8)
            nc.vector.tensor_scalar_mul(ex2, s2, inv_n)
            m2 = tp.tile([1, G], f32, tag="small", bufs=8)
            nc.vector.tensor_mul(m2, mean, mean)
            var = tp.tile([1, G], f32, tag="small", bufs=8)
            nc.vector.tensor_sub(var, ex2, m2)
            sd = tp.tile([1, G], f32, tag="small", bufs=8)
            eps_t = tp.tile([1, 1], f32, tag="small", bufs=8)
            nc.vector.memset(eps_t, float(EPS))
            nc.scalar.activation(out=sd, in_=var, func=AF.Sqrt, bias=eps_t)
            rstd = tp.tile([1, G], f32, tag="small", bufs=8)
            nc.vector.reciprocal(rstd, sd)

            # mean/rstd [1,32] -> columns [32,1] -> expand to channel rows
            gcol_ps = psp.tile([G, 2], f32, tag="ps", bufs=2)
            nc.tensor.matmul(gcol_ps[:, 0:1], rstd, one_f, start=True, stop=True)
            nc.tensor.matmul(gcol_ps[:, 1:2], mean, one_f, start=True, stop=True)
            gcol = tp.tile([G, 2], f32, tag="small", bufs=8)
            nc.vector.tensor_copy(gcol, gcol_ps)
            rstd_e_ps = psp.tile([1, C], f32, tag="ps", bufs=2)
            nc.tensor.matmul(rstd_e_ps, gcol[:, 0:1], eg_sb, start=True, stop=True)
            a_row = tp.tile([1, C], f32, tag="row", bufs=4)
            nc.vector.tensor_mul(a_row, rstd_e_ps, gs_row)
            mean_e_ps = psp.tile([1, C], f32, tag="ps", bufs=2)
            nc.tensor.matmul(mean_e_ps, gcol[:, 1:2], eg_sb, start=True, stop=True)
            mb = tp.tile([1, C], f32, tag="row", bufs=4)
            nc.vector.tensor_mul(mb, mean_e_ps, a_row)
            b_row = tp.tile([1, C], f32, tag="row", bufs=4)
            nc.vector.tensor_sub(b_row, gb_row, mb)
            # S-scaled A column (folded into wq/wk/wv rows)
            a16_row = tp.tile([1, C], f32, tag="row", bufs=4)
            nc.vector.tensor_scalar_mul(a16_row, a_row, float(S))
            aT = pp.tile([PT, CCH], f32, tag="aT")
            row_to_col(a16_row, aT, "aT")
            bT = pp.tile([PT, CCH], f32, tag="bT")
            row_to_col(b_row, bT, "bT")
            bT_bf = pp.tile([PT, CCH], bf16, tag="bT_bf")
            nc.vector.tensor_copy(bT_bf, bT)

            # bf16 copies of wq/wk/wv for the (tiny) B@w bias-fold matmuls —
            # bf16 matmuls run 4x faster than fp32 ones
            wfb = {}
            for n in "qkvo":
                wfb[n] = tp.tile([PT, CCH, C], bf16, tag="wfb", bufs=4, name="wfb_" + n)
                for j in range(CCH):
                    nc.vector.tensor_copy(wfb[n][:, j, :], wf[n][:, j, :])

            # ---- phase WP: fold GN into weights & biases ----
            # b'q/b'k = S*(B @ w + b), computed as rows then moved to columns
            bq_f = pp.tile([PT, CCH], f32, tag="bq_f")
            bk_f = pp.tile([PT, CCH], f32, tag="bk_f")
            for n, bias_row, out_t in (("q", bq_row, bq_f), ("k", bk_row, bk_f)):
                psb = psp.tile([1, C], f32, tag="ps", bufs=2, name="psb_" + n)
                for j in range(CCH):
                    nc.tensor.matmul(
                        psb,
                        bT_bf[:, j : j + 1],
                        wfb[n][:, j, :],
                        start=(j == 0),
                        stop=(j == CCH - 1),
                    )
                bp_row = tp.tile([1, C], f32, tag="row", bufs=4, name="bp_" + n)
                nc.vector.tensor_add(bp_row, psb, bias_row)
                bp16_row = tp.tile([1, C], f32, tag="row", bufs=4, name="bp16_" + n)
                nc.vector.tensor_scalar_mul(bp16_row, bp_row, float(S))
                row_to_col(bp16_row, out_t, "b" + n)
            # b'v as a row [1, 512] (bias enters V via ones-row matmul)
            psv = psp.tile([1, C], f32, tag="pz")
            for j in range(CCH):
                nc.tensor.matmul(
                    psv,
                    bT_bf[:, j : j + 1],
                    wfb["v"][:, j, :],
                    start=(j == 0),
                    stop=(j == CCH - 1),
                )
            bvp = tp.tile([1, C], f32, tag="row", bufs=4)
            nc.vector.tensor_add(bvp, psv, bv_row)
            # V's bias commutes through the softmax average EXACTLY:
            # softmax@(v + b'v) @ wo = softmax@v@wo + b'v@wo, so fold
            # b'v@wo into the output bias and keep V bias-free.
            bvpT = pp.tile([PT, CCH], f32, tag="bvpT")
            row_to_col(bvp, bvpT, "bvpT")
            bvpT_bf = pp.tile([PT, CCH], bf16, tag="bvpT_bf")
            nc.vector.tensor_copy(bvpT_bf, bvpT)
            bvo_ps = psp.tile([1, C], f32, tag="pz", name="bvo_ps")
            for j in range(CCH):
                nc.tensor.matmul(
                    bvo_ps,
                    bvpT_bf[:, j : j + 1],
                    wfb["o"][:, j, :],
                    start=(j == 0),
                    stop=(j == CCH - 1),
                )
            bvo_bf = tp.tile([1, C], bf16, tag="row2", bufs=2)
            nc.vector.tensor_copy(bvo_bf, bvo_ps)
            bvo_bc_ps = psp.tile([PT, C], f32, tag="ps", bufs=2)
            nc.tensor.matmul(bvo_bc_ps, ones_row, bvo_bf, start=True, stop=True)
            # effective output bias: bo + b'v@wo, broadcast across partitions
            bo2 = pp.tile([PT, C], f32, tag="bo2")
            nc.vector.tensor_add(bo2, bvo_bc_ps, bo_bc)

            # scale+cast weights: wq/k/v rows scaled by S*A (per input channel)
            for n in "qkv":
                for j in range(CCH):
                    nc.vector.tensor_scalar_mul(
                        wb[n][:, j, :], wf[n][:, j, :], aT[:, j : j + 1]
                    )
            # wo in fp8 scaled by 64/S: the O~ accumulator is scaled by 1/64
            # in the PSUM->fp8 copy, so the product keeps the same scale
            for j in range(CCH):
                nc.vector.tensor_scalar_mul(
                    wb["o"][:, j, :], wf["o"][:, j, :], 64.0 / float(S)
                )

            # ---- phase P: projections (fp8 DoubleRow, 2x256-deep chains).
            # Q^T/K^T PSUM tiles drain through ACT (Identity + bias column);
            # V drains through DVE (bias varies along the free dim). ----
            for m in range(CCH):
                for lb in range(LQ // NB):
                    ps = psp.tile([PT, NB], f32, tag="po", bufs=4)
                    for jp in range(CCH // 2):
                        nc.tensor.matmul(
                            ps,
                            wb["q"][:, 2 * jp : 2 * jp + 2, m * PT : (m + 1) * PT],
                            xT[:, 2 * jp : 2 * jp + 2, lb * NB : (lb + 1) * NB],
                            start=(jp == 0),
                            stop=(jp == CCH // 2 - 1),
                            perf_mode=DR,
                        )
                    # alternate drains between ACT and DVE: either engine
                    # alone is slower than the PE producing the tiles
                    if (m * (LQ // NB) + lb) % 2 == 0:
                        nc.scalar.activation(
                            out=qT[:, m, lb * NB : (lb + 1) * NB],
                            in_=ps,
                            func=AF.Identity,
                            bias=bq_f[:, m : m + 1],
                        )
                    else:
                        nc.vector.tensor_scalar_add(
                            qT[:, m, lb * NB : (lb + 1) * NB], ps, bq_f[:, m : m + 1]
                        )
            for m in range(CCH):
                for lb in range(L // NB):
                    ps = psp.tile([PT, NB], f32, tag="po", bufs=4)
                    for jp in range(CCH // 2):
                        nc.tensor.matmul(
                            ps,
                            wb["k"][:, 2 * jp : 2 * jp + 2, m * PT : (m + 1) * PT],
                            xT[:, 2 * jp : 2 * jp + 2, lb * NB : (lb + 1) * NB],
                            start=(jp == 0),
                            stop=(jp == CCH // 2 - 1),
                            perf_mode=DR,
                        )
                    if (m * (L // NB) + lb) % 2 == 0:
                        nc.scalar.activation(
                            out=kT[:, m, lb * NB : (lb + 1) * NB],
                            in_=ps,
                            func=AF.Identity,
                            bias=bk_f[:, m : m + 1],
                        )
                    else:
                        nc.vector.tensor_scalar_add(
                            kT[:, m, lb * NB : (lb + 1) * NB], ps, bk_f[:, m : m + 1]
                        )
            # V natural [s, c] for all rows; bias-free (folded into bo2),
            # so the drains are plain copies alternating ACT/DVE
            for t in range(NT):
                ps = psp.tile([PT, NB], f32, tag="po", bufs=4)
                for jp in range(CCH // 2):
                    nc.tensor.matmul(
                        ps,
                        xT[:, 2 * jp : 2 * jp + 2, t * PT : (t + 1) * PT],
                        wb["v"][:, 2 * jp : 2 * jp + 2, :],
                        start=(jp == 0),
                        stop=(jp == CCH // 2 - 1),
                        perf_mode=DR,
                    )
                if t % 2 == 0:
                    nc.scalar.activation(
                        out=v_sb[:, t, :], in_=ps, func=AF.Identity, bias=0.0
                    )
                else:
                    nc.vector.tensor_copy(v_sb[:, t, :], ps)

            # ---- phase A: attention, software-pipelined one key-pair ahead
            # across the four 512-wide query blocks ----
            ctx = {}

            def ensure_ctx(lb):
                if lb in ctx:
                    return ctx[lb]
                zps = psp.tile([1, NB], f32, tag="pz", name=f"zps{lb}")
                ops = [
                    psp.tile([PT, NB], f32, tag="po", bufs=4, name=f"ops{m}")
                    for m in range(CCH)
                ]
                # pre-create the epilogue PSUM tiles so the po-pool FIFO
                # order is ops(lb), yps(lb), ops(lb+1) — creating yps later
                # would deadlock the pool behind the next block's ops
                yps_l = [
                    psp.tile([PT, NB], f32, tag="po", bufs=4, name=f"yps{s_}")
                    for s_ in range(NB // PT)
                ]
                xr4 = tp.tile([PT, 4, C], bf16, tag="xr4", bufs=2, name=f"xr4_{lb}")
                nc.sync.dma_start(
                    out=xr4,
                    in_=x_d[lb * NB : (lb + 1) * NB, :].rearrange(
                        "(i p) c -> p i c", p=PT
                    ),
                )
                ctx[lb] = dict(zps=zps, ops=ops, yps=yps_l, xr4=xr4, a={}, zp={}, z4={})
                return ctx[lb]

            # global half-step score emitter: sh = lb*NT + st, rotating the
            # scores PSUM through ps,ps,py for an effective 3-deep buffer
            def emit_score_half(sh):
                lb, st = divmod(sh, NT)
                p = st // 2
                half = st % 2
                c = ensure_ctx(lb)
                if half == 0:
                    c["a"][p] = tp.tile(
                        [PT, 2, NB], f8, tag="a_t", bufs=3, name=f"ap{p % 3}"
                    )
                a_pair = c["a"][p]
                sps = psp.tile(
                    [PT, NB], f32, tag=("py" if sh % 3 == 2 else "ps"),
                    bufs=(1 if sh % 3 == 2 else 2), name=f"sps{sh % 3}",
                )
                for jp in range(CCH // 2):
                    nc.tensor.matmul(
                        sps,
                        kT[:, 2 * jp : 2 * jp + 2, st * PT : (st + 1) * PT],
                        qT[:, 2 * jp : 2 * jp + 2, lb * NB : (lb + 1) * NB],
                        start=(jp == 0),
                        stop=(jp == CCH // 2 - 1),
                        perf_mode=DR,
                    )
                nc.scalar.activation(
                    out=a_pair[:, half, :],
                    in_=sps,
                    func=AF.Exp,
                    scale=SCALE / float(S * S),
                    bias=expb_t,
                )

            def emit_epilogue(lb):
                c = ctx[lb]
                # Z row -> [128, 4] columns, then cheap per-partition 1/Z
                zrow = tp.tile([1, NB], f32, tag="row", bufs=4, name=f"zrow{lb}")
                nc.vector.tensor_copy(zrow, c["zps"])
                zTr = tp.tile([PT, NB // PT], f32, tag="zTr", bufs=2)
                row_to_col(zrow, zTr, f"zT{lb}")
                zT = tp.tile([PT, NB // PT], f32, tag="zT", bufs=2)
                nc.vector.reciprocal(zT, zTr)
                # O~ accumulators -> fp8 pairs (scaled 1/64) for DR out-proj
                obf8 = []
                for mp in range(CCH // 2):
                    ot = tp.tile([PT, 2, NB], f8, tag="obf", bufs=2, name=f"obf{mp}")
                    nc.scalar.activation(
                        out=ot[:, 0, :], in_=c["ops"][2 * mp],
                        func=AF.Identity, scale=1.0 / 64.0, bias=0.0,
                    )
                    nc.vector.tensor_scalar_mul(
                        ot[:, 1, :], c["ops"][2 * mp + 1], 1.0 / 64.0
                    )
                    obf8.append(ot)
                # final projection; normalize by 1/Z and add bias+residual
                for sub in range(NB // PT):
                    t = lb * (NB // PT) + sub
                    yps = c["yps"][sub]
                    for mp in range(CCH // 2):
                        nc.tensor.matmul(
                            yps,
                            obf8[mp][:, :, sub * PT : (sub + 1) * PT],
                            wb["o"][:, 2 * mp : 2 * mp + 2, :],
                            start=(mp == 0),
                            stop=(mp == CCH // 2 - 1),
                            perf_mode=DR,
                        )
                    xrb = tp.tile([PT, C], f32, tag="xrb", bufs=2)
                    nc.vector.tensor_add(xrb, c["xr4"][:, sub, :], bo2)
                    yt = tp.tile([PT, C], f32, tag="yt", bufs=2)
                    nc.vector.scalar_tensor_tensor(
                        out=yt,
                        in0=yps,
                        scalar=zT[:, sub : sub + 1],
                        in1=xrb,
                        op0=mybir.AluOpType.mult,
                        op1=mybir.AluOpType.add,
                    )
                    nc.sync.dma_start(out=y_d[t * PT : (t + 1) * PT, :], in_=yt)
                del ctx[lb]

            LOOKAHEAD = 3  # half-steps of score emission ahead of AV
            total_sh = NLB * NT
            sh = 0
            while sh < min(2 + LOOKAHEAD, total_sh):
                emit_score_half(sh)
                sh += 1
            for gi in range(NLB * NP):
                lb, p = divmod(gi, NP)
                c = ctx[lb]
                while sh < min(2 * (gi + 1) + LOOKAHEAD, total_sh):
                    emit_score_half(sh)
                    sh += 1
                a_pair = c["a"].pop(p)
                for m in range(CCH):
                    nc.tensor.matmul(
                        c["ops"][m],
                        v_sb[:, 2 * p : 2 * p + 2, m * PT : (m + 1) * PT],
                        a_pair,
                        start=(p == 0),
                        stop=(p == NP - 1),
                        perf_mode=DR,
                    )
                # Z via a ones DoubleRow matmul on the same a_pair the AV
                # just consumed — no DVE pair-sums, no deferral chains
                nc.tensor.matmul(
                    c["zps"],
                    ones82[:, :, 0:1],
                    a_pair,
                    start=(p == 0),
                    stop=(p == NP - 1),
                    perf_mode=DR,
                )
                if p == NP - 1:
                    emit_epilogue(lb)

    nc.compile()
    return nc


_NC_CACHE = None


def _get_program():
    global _NC_CACHE
    if _NC_CACHE is None:
        _NC_CACHE = build_program()
    return _NC_CACHE


def make_in_maps(inputs):
    import ml_dtypes

    hs = np.ascontiguousarray(np.asarray(inputs["hidden_states"], np.float32))
    ws = {n: np.ascontiguousarray(np.asarray(inputs["w" + n], np.float32)) for n in "qkvo"}
    bs = {n: np.ascontiguousarray(np.asarray(inputs["b" + n], np.float32)) for n in "qkvo"}
    gsc = np.ascontiguousarray(np.asarray(inputs["gn_scale"], np.float32))
    gbi = np.ascontiguousarray(np.asarray(inputs["gn_bias"], np.float32))
    eye = np.eye(PT, dtype=ml_dtypes.bfloat16)
    eg = np.zeros((G, C), np.float32)
    eg[np.arange(C) // GS, np.arange(C)] = 1.0
    in_maps = []
    for core in range(NCORES):
        b, h = core // 2, core % 2
        xb = hs[b].reshape(L, C)
        x_roll = np.ascontiguousarray(
            np.roll(xb, -h * LQ, axis=0).astype(ml_dtypes.bfloat16)
        )
        m = {"x": x_roll, "gn_scale": gsc, "gn_bias": gbi, "egrp": eg, "eye": eye}
        for n in "qkvo":
            m["w" + n] = ws[n]
            m["b" + n] = bs[n]
        in_maps.append(m)
    return in_maps


def assemble(results):
    out = np.empty((B, L, C), np.float32)
    for core in range(NCORES):
        b, h = core // 2, core % 2
        out[b, h * LQ : (h + 1) * LQ] = results[core]["y"]
    return out.reshape(B, HH, WW, C)


def kernel(**inputs):
    from concourse.bass_utils import run_bass_kernel_spmd

    nc = _get_program()
    in_maps = make_in_maps(inputs)
    res = run_bass_kernel_spmd(nc, in_maps, list(range(NCORES)))
    return assemble(res.results)


if __name__ == "__main__":
    rng = np.random.default_rng(0)
    s = 1.0 / np.sqrt(C)
    inputs = {
        "hidden_states": rng.standard_normal((B, HH, WW, C), np.float32),
        "gn_scale": np.ones(C, np.float32),
        "gn_bias": np.zeros(C, np.float32),
    }
    for n in "qkvo":
        inputs["w" + n] = (rng.standard_normal((C, C)) * s).astype(np.float32)
        inputs["b" + n] = np.zeros(C, np.float32)
    out = kernel(**inputs)
    print(out.shape, out.dtype)


# revision 39
# speedup vs baseline: 1.0182x; 1.0182x over previous
"""Trainium2 Bass kernel for GroupNorm + single-head self-attention block.

Reference computation (per batch b):
    xn = GroupNorm(x; 32 groups over (L, C/32)) * gn_scale + gn_bias
    q, k, v = xn@wq+bq, xn@wk+bk, xn@wv+bv
    out = softmax(q k^T / sqrt(C)) v @ wo + bo + x

Sharding: 8 cores = 4 batches x 2 query-halves. Each core receives its
batch's [L=4096, C=512] slice ROTATED so that its 2048 query rows are
always rows 0..2047 (attention and GN stats are invariant to key/value
ordering), which keeps the program SPMD-identical across cores.

On-core dataflow (fp8 DoubleRow matmuls everywhere heavy, fp32 accum):
  - x is pre-cast to bf16 on the host (stats, transposes and the
    residual all tolerate bf16), halving the input stream to 4MB.
  - GN is folded into the projections: A[c]=rstd[g]*gn_scale[c],
    B[c]=gn_bias[c]-mean[g]*A[c]; wq'=S*A (.) wq (row scaling),
    b'q = S*(B@wq + bq), with S=16 a power-of-2 pre-scale that keeps
    the fp8 weights out of the subnormal range. wo'=wo*64/S with the
    O~ accumulator scaled by 1/64 in its PSUM->fp8 copy.
  - Phase X: x streams in bf16, is cast to fp8 (xball, SBUF-resident);
    GN stats accumulate via fp8 DoubleRow ones-matmuls (x and fp8
    squares from ACT). The 128 PE transposes of xball -> xT [C, L] are
    emitted AFTER the stats loop so they fill the PE while the serial
    GN-stats chain (DVE/ACT) runs, keeping the HAM clock-gate warm.
  - Q^T/K^T PSUM tiles drain alternately through ACT (Identity +
    per-partition bias column) and DVE, because either engine alone is
    slower than the PE producing the tiles. V is computed BIAS-FREE:
    softmax@(v+b)@wo = softmax@v@wo + b@wo, so b'v@wo folds exactly
    into the output bias and V's drains are plain ACT/DVE copies.
  - Attention is software-pipelined one key-pair ahead ACROSS query
    blocks: the PE queue order is scores(next pair), zsum(deferred),
    AV(current), with each block's epilogue emitted after the next
    block's first scores, so the strict-FIFO engine queue never
    head-of-line blocks on ACT exps or DVE drains.
  - Row sums Z come from ones-matmuls over DVE pair-sums of exp tiles,
    deferred 2 pairs; 1/Z is taken on a [128,4] column layout after a
    PE transpose of the Z row; the final DR projection is scaled by
    1/Z and fused with bias + residual in one DVE op.
"""

import sys

sys.path.insert(0, "/opt/trn_rl_repo")

import numpy as np

B, HH, WW, C = 4, 64, 64, 512
L = HH * WW          # 4096
G = 32               # groups
GS = C // G          # 16 channels per group
EPS = 1e-6
NCORES = 8
LQ = L // 2          # 2048 query rows per core
PT = 128             # partition tile
NT = L // PT         # 32 row tiles
CCH = C // PT        # 4 channel chunks
NB = 512             # matmul moving-free block
NP = NT // 2         # 16 key pairs in attention
NLB = LQ // NB       # 4 query blocks
S = 16.0             # fp8 pre-scale folded into wq/wk/wv rows
SCALE = 1.0 / float(np.sqrt(C))
EXPB = -4.0          # constant shift inside exp (cancels in softmax)


def build_program():
    import concourse.bacc as bacc
    import concourse.bass as bass
    import concourse.mybir as mybir
    import concourse.tile as tile

    f32 = mybir.dt.float32
    bf16 = mybir.dt.bfloat16
    f8 = mybir.dt.float8e4
    AF = mybir.ActivationFunctionType
    DR = mybir.MatmulPerfMode.DoubleRow

    nc = bacc.Bacc(
        trn_type="TRN2",
        target_bir_lowering=False,
        debug=False,
        num_devices=NCORES,
    )

    x_d = nc.dram_tensor("x", [L, C], bf16, kind="ExternalInput").ap()
    gs_d = nc.dram_tensor("gn_scale", [C], f32, kind="ExternalInput").ap()
    gb_d = nc.dram_tensor("gn_bias", [C], f32, kind="ExternalInput").ap()
    w_d = {}
    b_d = {}
    for n in "qkvo":
        w_d[n] = nc.dram_tensor("w" + n, [C, C], f32, kind="ExternalInput").ap()
        b_d[n] = nc.dram_tensor("b" + n, [C], f32, kind="ExternalInput").ap()
    eg_d = nc.dram_tensor("egrp", [G, C], f32, kind="ExternalInput").ap()
    eye_d = nc.dram_tensor("eye", [PT, PT], bf16, kind="ExternalInput").ap()
    y_d = nc.dram_tensor("y", [LQ, C], f32, kind="ExternalOutput").ap()

    with tile.TileContext(nc) as tc:
        with (
            tc.tile_pool(name="persist", bufs=1) as pp,
            tc.tile_pool(name="trans", bufs=1) as tp,
            tc.tile_pool(name="dram", bufs=1, space="DRAM") as dp,
            tc.tile_pool(name="psum", bufs=1, space="PSUM") as psp,
        ):
            # ---- constants ----
            ones_col = pp.tile([PT, 1], bf16, tag="ones_col")
            nc.vector.memset(ones_col, 1.0)
            # fp8 DoubleRow ones for the stats matmuls; middle-dim byte
            # step must be 16-aligned, hence the padded [PT, 2, 16] tile
            ones82 = pp.tile([PT, 2, 16], f8, tag="ones82")
            nc.vector.memset(ones82, 1.0)
            ones_row = pp.tile([1, PT], bf16, tag="ones_row")
            nc.vector.memset(ones_row, 1.0)
            expb_t = pp.tile([PT, 1], f32, tag="expb")
            nc.vector.memset(expb_t, float(EXPB))
            one_f = pp.tile([1, 1], f32, tag="one_f")
            nc.vector.memset(one_f, 1.0)
            # warm-up burst: ~4us of tiny matmuls on constants while the
            # first x DMA is in flight, so the HAM clock-gate reaches
            # 8/8 before the real work starts
            warm_sb = pp.tile([PT, 64], bf16, tag="warm")
            nc.vector.memset(warm_sb, 1.0)
            wps = psp.tile([1, 64], f32, tag="ps", bufs=2, name="warm_ps")
            for _ in range(40):
                nc.tensor.matmul(wps, ones_col, warm_sb, start=True, stop=True)

            # nothing precedes the x stream on the sync ring
            eye_sb = pp.tile([PT, PT], bf16, tag="eye")
            nc.gpsimd.dma_start(out=eye_sb, in_=eye_d)
            # group->channel indicator matrix [G, C] (host-side constant)
            eg_sb = pp.tile([G, C], f32, tag="eg")
            nc.gpsimd.dma_start(out=eg_sb, in_=eg_d)

            # ---- persistent tensors ----
            xball = pp.tile([PT, NT, C], bf16, tag="xball")     # 4 MB
            xT = pp.tile([PT, CCH, L], f8, tag="xT")            # 2 MB
            qT = pp.tile([PT, CCH, LQ], f8, tag="qT")           # 1 MB
            kT = pp.tile([PT, CCH, L], f8, tag="kT")            # 2 MB
            v_sb = pp.tile([PT, NT, NB], f8, tag="v")           # 2 MB
            wb = {
                n: pp.tile([PT, CCH, C], f8, tag="wb" + n, name="wb_" + n)
                for n in "qkvo"
            }

            def row_to_col(row, out_sb, nm):
                """[1, n*128] row -> [128, n] column layout, via K=1 matmuls."""
                n = out_sb.shape[-1]
                ps = psp.tile(
                    [PT, n], f32, tag="ps", bufs=2, name="r2c_" + nm
                )
                for m_ in range(n):
                    nc.tensor.matmul(
                        ps[:, m_ : m_ + 1],
                        row[0:1, m_ * PT : (m_ + 1) * PT],
                        one_f,
                        start=True,
                        stop=True,
                    )
                nc.vector.tensor_copy(out_sb, ps)

            # ---- small bias/scale loads (gpsimd ring; off the x stream) ----
            bq_row = pp.tile([1, C], f32, tag="bq_row")
            nc.gpsimd.dma_start(out=bq_row, in_=b_d["q"].unsqueeze(0))
            bk_row = pp.tile([1, C], f32, tag="bk_row")
            nc.gpsimd.dma_start(out=bk_row, in_=b_d["k"].unsqueeze(0))
            bv_row = pp.tile([1, C], f32, tag="bv_row")
            nc.gpsimd.dma_start(out=bv_row, in_=b_d["v"].unsqueeze(0))
            bo_bc = pp.tile([PT, C], f32, tag="bo_bc")
            nc.gpsimd.dma_start(
                out=bo_bc, in_=b_d["o"].unsqueeze(0).broadcast_to([PT, C])
            )
            gs_row = pp.tile([1, C], f32, tag="gs_row")
            nc.gpsimd.dma_start(out=gs_row, in_=gs_d.unsqueeze(0))
            gb_row = pp.tile([1, C], f32, tag="gb_row")
            nc.gpsimd.dma_start(out=gb_row, in_=gb_d.unsqueeze(0))

            # ---- phase X: stream x (bf16), cast to fp8, GN stats ----
            sum_ps = psp.tile([1, C], f32, tag="pz")
            sq_ps = psp.tile([1, C], f32, tag="py")
            for t2 in range(NT // 2):
                xf4 = tp.tile([PT, 2, C], bf16, tag="xf4", bufs=3, name=f"xf4_{t2}")
                nc.sync.dma_start(
                    out=xf4,
                    in_=x_d[t2 * 2 * PT : (t2 + 1) * 2 * PT, :].rearrange(
                        "(i p) c -> p i c", p=PT
                    ),
                )
                for i in range(2):
                    t = 2 * t2 + i
                    nc.vector.tensor_copy(xball[:, t, :], xf4[:, i, :])
                    sq = tp.tile([PT, C], bf16, tag="sq", bufs=2)
                    nc.scalar.activation(out=sq, in_=xf4[:, i, :], func=AF.Square)
                    nc.tensor.matmul(
                        sum_ps, ones_col, xball[:, t, :],
                        start=(t == 0), stop=(t == NT - 1),
                    )
                    nc.tensor.matmul(
                        sq_ps, ones_col, sq,
                        start=(t == 0), stop=(t == NT - 1),
                    )

            # ---- weight loads: sync ring AFTER the x stream (FIFO keeps
            # them from competing with x for HBM bandwidth) ----
            wf = {}
            for n in "qkvo":
                wf[n] = tp.tile([PT, CCH, C], f32, tag="wf", bufs=3, name="wf_" + n)
                nc.sync.dma_start(
                    out=wf[n], in_=w_d[n].rearrange("(j p) c -> p j c", p=PT)
                )

            # ---- phase S (DVE/ACT part): emitted BEFORE the transposes so
            # these latency-critical small ops queue ahead of the bulk
            # xT copies in the DVE FIFO; it contains no PE instructions,
            # so the transposes below keep the PE busy meanwhile ----
            # ---- phase S: GN stats -> A,B rows -> bounce to [128,4] ----
            s1 = tp.tile([1, G], f32, tag="small", bufs=8)
            nc.vector.reduce_sum(
                out=s1,
                in_=sum_ps.rearrange("p (g d) -> p g d", g=G),
                axis=mybir.AxisListType.X,
            )
            s2 = tp.tile([1, G], f32, tag="small", bufs=8)
            nc.vector.reduce_sum(
                out=s2,
                in_=sq_ps.rearrange("p (g d) -> p g d", g=G),
                axis=mybir.AxisListType.X,
            )
            inv_n = 1.0 / float(L * GS)
            mean = tp.tile([1, G], f32, tag="small", bufs=8)
            nc.vector.tensor_scalar_mul(mean, s1, inv_n)
            ex2 = tp.tile([1, G], f32, tag="small", bufs=8)
            nc.vector.tensor_scalar_mul(ex2, s2, inv_n)
            m2 = tp.tile([1, G], f32, tag="small", bufs=8)
            nc.vector.tensor_mul(m2, mean, mean)
            var = tp.tile([1, G], f32, tag="small", bufs=8)
            nc.vector.tensor_sub(var, ex2, m2)
            sd = tp.tile([1, G], f32, tag="small", bufs=8)
            eps_t = tp.tile([1, 1], f32, tag="small", bufs=8)
            nc.vector.memset(eps_t, float(EPS))
            nc.scalar.activation(out=sd, in_=var, func=AF.Sqrt, bias=eps_t)
            rstd = tp.tile([1, G], f32, tag="small", bufs=8)
            nc.vector.reciprocal(rstd, sd)

            # ---- phase T: transposes, emitted after stats so they fill the
            # PE while the serial stats chain runs on DVE/ACT ----
            for t in range(NT):
                t_ps = psp.tile([PT, NB], bf16, tag="po", bufs=4, name=f"tps{t}")
                for j in range(CCH):
                    nc.tensor.transpose(
                        t_ps[:, j * PT : (j + 1) * PT],
                        xball[:, t, j * PT : (j + 1) * PT],
                        eye_sb,
                    )
                nc.vector.tensor_copy(
                    xT[:, :, t * PT : (t + 1) * PT],
                    t_ps.rearrange("p (j i) -> p j i", j=CCH),
                )

            # mean/rstd [1,32] -> columns [32,1] -> expand to channel rows
            gcol_ps = psp.tile([G, 2], f32, tag="ps", bufs=2)
            nc.tensor.matmul(gcol_ps[:, 0:1], rstd, one_f, start=True, stop=True)
            nc.tensor.matmul(gcol_ps[:, 1:2], mean, one_f, start=True, stop=True)
            gcol = tp.tile([G, 2], f32, tag="small", bufs=8)
            nc.vector.tensor_copy(gcol, gcol_ps)
            rstd_e_ps = psp.tile([1, C], f32, tag="ps", bufs=2)
            nc.tensor.matmul(rstd_e_ps, gcol[:, 0:1], eg_sb, start=True, stop=True)
            a_row = tp.tile([1, C], f32, tag="row", bufs=4)
            nc.vector.tensor_mul(a_row, rstd_e_ps, gs_row)
            mean_e_ps = psp.tile([1, C], f32, tag="ps", bufs=2)
            nc.tensor.matmul(mean_e_ps, gcol[:, 1:2], eg_sb, start=True, stop=True)
            mb = tp.tile([1, C], f32, tag="row", bufs=4)
            nc.vector.tensor_mul(mb, mean_e_ps, a_row)
            b_row = tp.tile([1, C], f32, tag="row", bufs=4)
            nc.vector.tensor_sub(b_row, gb_row, mb)
            # S-scaled A column (folded into wq/wk/wv rows)
            a16_row = tp.tile([1, C], f32, tag="row", bufs=4)
            nc.vector.tensor_scalar_mul(a16_row, a_row, float(S))
            aT = pp.tile([PT, CCH], f32, tag="aT")
            row_to_col(a16_row, aT, "aT")
            bT = pp.tile([PT, CCH], f32, tag="bT")
            row_to_col(b_row, bT, "bT")
            bT_bf = pp.tile([PT, CCH], bf16, tag="bT_bf")
            nc.vector.tensor_copy(bT_bf, bT)

            # bf16 copies of wq/wk/wv for the (tiny) B@w bias-fold matmuls —
            # bf16 matmuls run 4x faster than fp32 ones
            wfb = {}
            for n in "qkvo":
                wfb[n] = tp.tile([PT, CCH, C], bf16, tag="wfb", bufs=4, name="wfb_" + n)
                for j in range(CCH):
                    nc.vector.tensor_copy(wfb[n][:, j, :], wf[n][:, j, :])

            # ---- phase WP: fold GN into weights & biases ----
            # b'q/b'k = S*(B @ w + b), computed as rows then moved to columns
            bq_f = pp.tile([PT, CCH], f32, tag="bq_f")
            bk_f = pp.tile([PT, CCH], f32, tag="bk_f")
            for n, bias_row, out_t in (("q", bq_row, bq_f), ("k", bk_row, bk_f)):
                psb = psp.tile([1, C], f32, tag="ps", bufs=2, name="psb_" + n)
                for j in range(CCH):
                    nc.tensor.matmul(
                        psb,
                        bT_bf[:, j : j + 1],
                        wfb[n][:, j, :],
                        start=(j == 0),
                        stop=(j == CCH - 1),
                    )
                bp_row = tp.tile([1, C], f32, tag="row", bufs=4, name="bp_" + n)
                nc.vector.tensor_add(bp_row, psb, bias_row)
                bp16_row = tp.tile([1, C], f32, tag="row", bufs=4, name="bp16_" + n)
                nc.vector.tensor_scalar_mul(bp16_row, bp_row, float(S))
                row_to_col(bp16_row, out_t, "b" + n)
            # b'v as a row [1, 512] (bias enters V via ones-row matmul)
            psv = psp.tile([1, C], f32, tag="pz")
            for j in range(CCH):
                nc.tensor.matmul(
                    psv,
                    bT_bf[:, j : j + 1],
                    wfb["v"][:, j, :],
                    start=(j == 0),
                    stop=(j == CCH - 1),
                )
            bvp = tp.tile([1, C], f32, tag="row", bufs=4)
            nc.vector.tensor_add(bvp, psv, bv_row)
            # V's bias commutes through the softmax average EXACTLY:
            # softmax@(v + b'v) @ wo = softmax@v@wo + b'v@wo, so fold
            # b'v@wo into the output bias and keep V bias-free.
            bvpT = pp.tile([PT, CCH], f32, tag="bvpT")
            row_to_col(bvp, bvpT, "bvpT")
            bvpT_bf = pp.tile([PT, CCH], bf16, tag="bvpT_bf")
            nc.vector.tensor_copy(bvpT_bf, bvpT)
            bvo_ps = psp.tile([1, C], f32, tag="pz", name="bvo_ps")
            for j in range(CCH):
                nc.tensor.matmul(
                    bvo_ps,
                    bvpT_bf[:, j : j + 1],
                    wfb["o"][:, j, :],
                    start=(j == 0),
                    stop=(j == CCH - 1),
                )
            bvo_bf = tp.tile([1, C], bf16, tag="row2", bufs=2)
            nc.vector.tensor_copy(bvo_bf, bvo_ps)
            bvo_bc_ps = psp.tile([PT, C], f32, tag="ps", bufs=2)
            nc.tensor.matmul(bvo_bc_ps, ones_row, bvo_bf, start=True, stop=True)
            # effective output bias: bo + b'v@wo, broadcast across partitions
            bo2 = pp.tile([PT, C], f32, tag="bo2")
            nc.vector.tensor_add(bo2, bvo_bc_ps, bo_bc)

            # scale+cast weights: wq/k/v rows scaled by S*A (per input channel)
            for n in "qkv":
                for j in range(CCH):
                    nc.vector.tensor_scalar_mul(
                        wb[n][:, j, :], wf[n][:, j, :], aT[:, j : j + 1]
                    )
            # wo in fp8 scaled by 64/S: the O~ accumulator is scaled by 1/64
            # in the PSUM->fp8 copy, so the product keeps the same scale
            for j in range(CCH):
                nc.vector.tensor_scalar_mul(
                    wb["o"][:, j, :], wf["o"][:, j, :], 64.0 / float(S)
                )

            # ---- phase P: projections (fp8 DoubleRow, 2x256-deep chains).
            # Q^T/K^T PSUM tiles drain through ACT (Identity + bias column);
            # V drains through DVE (bias varies along the free dim). ----
            for m in range(CCH):
                for lb in range(LQ // NB):
                    ps = psp.tile([PT, NB], f32, tag="po", bufs=4)
                    for jp in range(CCH // 2):
                        nc.tensor.matmul(
                            ps,
                            wb["q"][:, 2 * jp : 2 * jp + 2, m * PT : (m + 1) * PT],
                            xT[:, 2 * jp : 2 * jp + 2, lb * NB : (lb + 1) * NB],
                            start=(jp == 0),
                            stop=(jp == CCH // 2 - 1),
                            perf_mode=DR,
                        )
                    # alternate drains between ACT and DVE: either engine
                    # alone is slower than the PE producing the tiles
                    if (m * (LQ // NB) + lb) % 2 == 0:
                        nc.scalar.activation(
                            out=qT[:, m, lb * NB : (lb + 1) * NB],
                            in_=ps,
                            func=AF.Identity,
                            bias=bq_f[:, m : m + 1],
                        )
                    else:
                        nc.vector.tensor_scalar_add(
                            qT[:, m, lb * NB : (lb + 1) * NB], ps, bq_f[:, m : m + 1]
                        )
            for m in range(CCH):
                for lb in range(L // NB):
                    ps = psp.tile([PT, NB], f32, tag="po", bufs=4)
                    for jp in range(CCH // 2):
                        nc.tensor.matmul(
                            ps,
                            wb["k"][:, 2 * jp : 2 * jp + 2, m * PT : (m + 1) * PT],
                            xT[:, 2 * jp : 2 * jp + 2, lb * NB : (lb + 1) * NB],
                            start=(jp == 0),
                            stop=(jp == CCH // 2 - 1),
                            perf_mode=DR,
                        )
                    if (m * (L // NB) + lb) % 2 == 0:
                        nc.scalar.activation(
                            out=kT[:, m, lb * NB : (lb + 1) * NB],
                            in_=ps,
                            func=AF.Identity,
                            bias=bk_f[:, m : m + 1],
                        )
                    else:
                        nc.vector.tensor_scalar_add(
                            kT[:, m, lb * NB : (lb + 1) * NB], ps, bk_f[:, m : m + 1]
                        )
            # V natural [s, c] for all rows; bias-free (folded into bo2),
            # so the drains are plain copies alternating ACT/DVE
            for t in range(NT):
                ps = psp.tile([PT, NB], f32, tag="po", bufs=4)
                for jp in range(CCH // 2):
                    nc.tensor.matmul(
                        ps,
                        xT[:, 2 * jp : 2 * jp + 2, t * PT : (t + 1) * PT],
                        wb["v"][:, 2 * jp : 2 * jp + 2, :],
                        start=(jp == 0),
                        stop=(jp == CCH // 2 - 1),
                        perf_mode=DR,
                    )
                if t % 2 == 0:
                    nc.scalar.activation(
                        out=v_sb[:, t, :], in_=ps, func=AF.Identity, bias=0.0
                    )
                else:
                    nc.vector.tensor_copy(v_sb[:, t, :], ps)

            # ---- phase A: attention, software-pipelined one key-pair ahead
            # across the four 512-wide query blocks ----
            ctx = {}

            def ensure_ctx(lb):
                if lb in ctx:
                    return ctx[lb]
                zps = psp.tile([1, NB], f32, tag="pz", name=f"zps{lb}")
                ops = [
                    psp.tile([PT, NB], f32, tag="po", bufs=4, name=f"ops{m}")
                    for m in range(CCH)
                ]
                # pre-create the epilogue PSUM tiles so the po-pool FIFO
                # order is ops(lb), yps(lb), ops(lb+1) — creating yps later
                # would deadlock the pool behind the next block's ops
                yps_l = [
                    psp.tile([PT, NB], f32, tag="po", bufs=4, name=f"yps{s_}")
                    for s_ in range(NB // PT)
                ]
                xr4 = tp.tile([PT, 4, C], bf16, tag="xr4", bufs=2, name=f"xr4_{lb}")
                nc.sync.dma_start(
                    out=xr4,
                    in_=x_d[lb * NB : (lb + 1) * NB, :].rearrange(
                        "(i p) c -> p i c", p=PT
                    ),
                )
                ctx[lb] = dict(zps=zps, ops=ops, yps=yps_l, xr4=xr4, a={}, zp={}, z4={})
                return ctx[lb]

            # global half-step score emitter: sh = lb*NT + st, rotating the
            # scores PSUM through ps,ps,py for an effective 3-deep buffer
            def emit_score_half(sh):
                lb, st = divmod(sh, NT)
                p = st // 2
                half = st % 2
                c = ensure_ctx(lb)
                if half == 0:
                    c["a"][p] = tp.tile(
                        [PT, 2, NB], f8, tag="a_t", bufs=3, name=f"ap{p % 3}"
                    )
                a_pair = c["a"][p]
                sps = psp.tile(
                    [PT, NB], f32, tag=("py" if sh % 3 == 2 else "ps"),
                    bufs=(1 if sh % 3 == 2 else 2), name=f"sps{sh % 3}",
                )
                for jp in range(CCH // 2):
                    nc.tensor.matmul(
                        sps,
                        kT[:, 2 * jp : 2 * jp + 2, st * PT : (st + 1) * PT],
                        qT[:, 2 * jp : 2 * jp + 2, lb * NB : (lb + 1) * NB],
                        start=(jp == 0),
                        stop=(jp == CCH // 2 - 1),
                        perf_mode=DR,
                    )
                nc.scalar.activation(
                    out=a_pair[:, half, :],
                    in_=sps,
                    func=AF.Exp,
                    scale=SCALE / float(S * S),
                    bias=expb_t,
                )

            def emit_epilogue(lb):
                c = ctx[lb]
                # Z row -> [128, 4] columns, then cheap per-partition 1/Z
                zrow = tp.tile([1, NB], f32, tag="row", bufs=4, name=f"zrow{lb}")
                nc.vector.tensor_copy(zrow, c["zps"])
                zTr = tp.tile([PT, NB // PT], f32, tag="zTr", bufs=2)
                row_to_col(zrow, zTr, f"zT{lb}")
                zT = tp.tile([PT, NB // PT], f32, tag="zT", bufs=2)
                nc.vector.reciprocal(zT, zTr)
                # O~ accumulators -> fp8 pairs (scaled 1/64) for DR out-proj
                obf8 = []
                for mp in range(CCH // 2):
                    ot = tp.tile([PT, 2, NB], f8, tag="obf", bufs=2, name=f"obf{mp}")
                    nc.scalar.activation(
                        out=ot[:, 0, :], in_=c["ops"][2 * mp],
                        func=AF.Identity, scale=1.0 / 64.0, bias=0.0,
                    )
                    nc.vector.tensor_scalar_mul(
                        ot[:, 1, :], c["ops"][2 * mp + 1], 1.0 / 64.0
                    )
                    obf8.append(ot)
                # final projection; normalize by 1/Z and add bias+residual
                for sub in range(NB // PT):
                    t = lb * (NB // PT) + sub
                    yps = c["yps"][sub]
                    for mp in range(CCH // 2):
                        nc.tensor.matmul(
                            yps,
                            obf8[mp][:, :, sub * PT : (sub + 1) * PT],
                            wb["o"][:, 2 * mp : 2 * mp + 2, :],
                            start=(mp == 0),
                            stop=(mp == CCH // 2 - 1),
                            perf_mode=DR,
                        )
                    xrb = tp.tile([PT, C], f32, tag="xrb", bufs=2)
                    nc.vector.tensor_add(xrb, c["xr4"][:, sub, :], bo2)
                    yt = tp.tile([PT, C], f32, tag="yt", bufs=2)
                    nc.vector.scalar_tensor_tensor(
                        out=yt,
                        in0=yps,
                        scalar=zT[:, sub : sub + 1],
                        in1=xrb,
                        op0=mybir.AluOpType.mult,
                        op1=mybir.AluOpType.add,
                    )
                    nc.sync.dma_start(out=y_d[t * PT : (t + 1) * PT, :], in_=yt)
                del ctx[lb]

            LOOKAHEAD = 3  # half-steps of score emission ahead of AV
            total_sh = NLB * NT
            sh = 0
            while sh < min(2 + LOOKAHEAD, total_sh):
                emit_score_half(sh)
                sh += 1
            for gi in range(NLB * NP):
                lb, p = divmod(gi, NP)
                c = ctx[lb]
                while sh < min(2 * (gi + 1) + LOOKAHEAD, total_sh):
                    emit_score_half(sh)
                    sh += 1
                a_pair = c["a"].pop(p)
                for m in range(CCH):
                    nc.tensor.matmul(
                        c["ops"][m],
                        v_sb[:, 2 * p : 2 * p + 2, m * PT : (m + 1) * PT],
                        a_pair,
                        start=(p == 0),
                        stop=(p == NP - 1),
                        perf_mode=DR,
                    )
                # Z via a ones DoubleRow matmul on the same a_pair the AV
                # just consumed — no DVE pair-sums, no deferral chains
                nc.tensor.matmul(
                    c["zps"],
                    ones82[:, :, 0:1],
                    a_pair,
                    start=(p == 0),
                    stop=(p == NP - 1),
                    perf_mode=DR,
                )
                if p == NP - 1:
                    emit_epilogue(lb)

    nc.compile()
    return nc


_NC_CACHE = None


def _get_program():
    global _NC_CACHE
    if _NC_CACHE is None:
        _NC_CACHE = build_program()
    return _NC_CACHE


def make_in_maps(inputs):
    import ml_dtypes

    hs = np.ascontiguousarray(np.asarray(inputs["hidden_states"], np.float32))
    ws = {n: np.ascontiguousarray(np.asarray(inputs["w" + n], np.float32)) for n in "qkvo"}
    bs = {n: np.ascontiguousarray(np.asarray(inputs["b" + n], np.float32)) for n in "qkvo"}
    gsc = np.ascontiguousarray(np.asarray(inputs["gn_scale"], np.float32))
    gbi = np.ascontiguousarray(np.asarray(inputs["gn_bias"], np.float32))
    eye = np.eye(PT, dtype=ml_dtypes.bfloat16)
    eg = np.zeros((G, C), np.float32)
    eg[np.arange(C) // GS, np.arange(C)] = 1.0
    in_maps = []
    for core in range(NCORES):
        b, h = core // 2, core % 2
        xb = hs[b].reshape(L, C)
        x_roll = np.ascontiguousarray(
            np.roll(xb, -h * LQ, axis=0).astype(ml_dtypes.bfloat16)
        )
        m = {"x": x_roll, "gn_scale": gsc, "gn_bias": gbi, "egrp": eg, "eye": eye}
        for n in "qkvo":
            m["w" + n] = ws[n]
            m["b" + n] = bs[n]
        in_maps.append(m)
    return in_maps


def assemble(results):
    out = np.empty((B, L, C), np.float32)
    for core in range(NCORES):
        b, h = core // 2, core % 2
        out[b, h * LQ : (h + 1) * LQ] = results[core]["y"]
    return out.reshape(B, HH, WW, C)


def kernel(**inputs):
    from concourse.bass_utils import run_bass_kernel_spmd

    nc = _get_program()
    in_maps = make_in_maps(inputs)
    res = run_bass_kernel_spmd(nc, in_maps, list(range(NCORES)))
    return assemble(res.results)


if __name__ == "__main__":
    rng = np.random.default_rng(0)
    s = 1.0 / np.sqrt(C)
    inputs = {
        "hidden_states": rng.standard_normal((B, HH, WW, C), np.float32),
        "gn_scale": np.ones(C, np.float32),
        "gn_bias": np.zeros(C, np.float32),
    }
    for n in "qkvo":
        inputs["w" + n] = (rng.standard_normal((C, C)) * s).astype(np.float32)
        inputs["b" + n] = np.zeros(C, np.float32)
    out = kernel(**inputs)
    print(out.shape, out.dtype)
